# revision 2
# baseline (speedup 1.0000x reference)
"""NeuralMeshFlow Trainium2 kernel (fp8 DoubleRow edition).

Strategy
--------
Shard the flattened (B=4, N=2562) = 10248 points across 8 cores: core c gets
half of batch c//2 (1281 points, padded to 1284 = 3*428).  All heavy compute
(96 MLP evals: 6 NODE blocks x 4 RK4 steps x 4 stages) runs on-device in one
SPMD NEFF; tiny conditioning math (cf vectors, AdaIN scale MLPs, initial
AdaIN) runs on host.

Device layout: activations are transposed — channels on SBUF partitions
(4 octiles x 128), points on the free dim (3 tiles x 428).  Matmuls use
out^T = lhsT.T @ rhs with lhsT = weight tiles in natural [in, out] layout.

The two H=512 x H=512 layers (L2, L3) and the H->3 output layer (L4) run in
fp8e4m3 with perf_mode=DoubleRow: lhsT [128, 2, M] packs two K-octiles per
PE cell, rhs streams [128, 2, N], halving the PE streaming time for the
K=512 contractions.  Activations h1/h2/h3 are stored fp8 (rel-err budget
verified off-line: ~2.3e-3 vs the 2e-2 gate).  L1 (K<=51) and the combine
stay bf16 — DoubleRow only pays off on the K dim.

RK4 is folded into the first matmul: each eval's effective input
x + sum_j C[e][j] k_j is never materialized; instead the rhs is the stacked
state tile [x; k_0..k_14] (partitions) and the lhsT is a host-precomputed
stack [W1; C[e][0] W1; ...].  The per-block combine x += sum_j D[j] k_j is a
single K=51 matmul against a striped coefficient matrix.

AdaIN means between DeformBlocks use a per-core partial sum (DVE reduce) and
a pairwise (cores 2b, 2b+1) AllReduce of 12 bytes.
"""

import numpy as np
import ml_dtypes

BF = ml_dtypes.bfloat16
F8 = ml_dtypes.float8_e4m3     # TRN FP8_EXP4: max +-240

B = 4
N_FULL = 2562
HALF = 1281          # points per core (2562 / 2)
P = 1284             # padded points per core (3 * 428)
TSZ = 428            # point-tile size (streamed)
TSW = 432            # point-tile stride in h tiles (16B-aligned for DoubleRow)
NT = 3               # point tiles
NBLK = 6             # NODE blocks
EV = 16              # dyn evals per block (4 RK4 steps x 4 stages)
KMAX = 3 * (1 + EV - 1) + 3   # 51 state rows: x (3) + 16 k's (48)
H = 512
TIME, N_STEPS = 0.2, 4
DT = TIME / N_STEPS

REPLICA_GROUPS = [[0, 1], [2, 3], [4, 5], [6, 7]]

TRACE = False            # set by test harness to capture an NTFF profile
LAST_RESULTS = None      # BassKernelResults of the last run (for profiling)

_CACHE = {}


def _rk4_coeffs():
    """C[e][j]: coefficient of k_j in eval e's input; Dfin[j]: coefficient in
    the final per-block combine  x_next = x + sum_j Dfin[j] k_j."""
    C = np.zeros((EV, EV), np.float64)
    Dcur = np.zeros(EV, np.float64)
    for s in range(4):
        e0 = 4 * s
        C[e0] = Dcur
        C[e0 + 1] = Dcur; C[e0 + 1][e0] = DT / 2
        C[e0 + 2] = Dcur; C[e0 + 2][e0 + 1] = DT / 2
        C[e0 + 3] = Dcur; C[e0 + 3][e0 + 2] = DT
        Dcur = Dcur.copy()
        for j, w in zip(range(e0, e0 + 4), (DT / 6, DT / 3, DT / 3, DT / 6)):
            Dcur[j] += w
    return C.astype(np.float32), Dcur.astype(np.float32)


def _build_bass(zero_bias):
    import concourse.bass as bass
    import concourse.tile as tile
    from concourse import bacc, mybir

    f32 = mybir.dt.float32
    bf16 = mybir.dt.bfloat16
    fp8 = mybir.dt.float8e4
    Alu = mybir.AluOpType
    Act = mybir.ActivationFunctionType
    DR = mybir.MatmulPerfMode.DoubleRow
    ts = bass.ts

    nc = bacc.Bacc("TRN2", target_bir_lowering=False, debug=False, num_devices=8)

    x0_d = nc.dram_tensor("x0", [3, P], f32, kind="ExternalInput").ap()
    w1s_d = nc.dram_tensor("w1s", [KMAX, NBLK, EV, 4, 128], bf16, kind="ExternalInput").ap()
    # DoubleRow packs: [p, b, l, kp, m, i, mm] and [p, b, kp, i, 16]
    w23_d = nc.dram_tensor("w23", [128, NBLK, 2, 2, 4, 2, 128], fp8, kind="ExternalInput").ap()
    w4_d = nc.dram_tensor("w4", [128, NBLK, 2, 2, 16], fp8, kind="ExternalInput").ap()
    b123_d = nc.dram_tensor("b123", [128, NBLK * 12], f32, kind="ExternalInput").ap()
    b4_d = nc.dram_tensor("b4", [3, NBLK], f32, kind="ExternalInput").ap()
    cf_d = nc.dram_tensor("cf", [128, NBLK * 4], f32, kind="ExternalInput").ap()
    dfin_d = nc.dram_tensor("dfin", [KMAX, 3], bf16, kind="ExternalInput").ap()
    adain_d = nc.dram_tensor("adain", [3, 9], f32, kind="ExternalInput").ap()
    out_d = nc.dram_tensor("out", [3, 3, P], f32, kind="ExternalOutput").ap()

    with tile.TileContext(nc) as tc:
        with (
            tc.tile_pool(name="consts", bufs=1) as consts,
            tc.tile_pool(name="wpool", bufs=2) as wpool,
            tc.tile_pool(name="hpool", bufs=2) as hpool,
            tc.tile_pool(name="spool", bufs=1) as spool,
            tc.tile_pool(name="pspool", bufs=2, space="PSUM") as pspool,
            tc.tile_pool(name="dpool", bufs=1, space="DRAM") as dpool,
        ):
            # ---- constants ----
            b123_sb = consts.tile([128, NBLK * 12], f32)
            nc.sync.dma_start(out=b123_sb, in_=b123_d)
            b4_sb = consts.tile([3, NBLK], f32)
            nc.sync.dma_start(out=b4_sb, in_=b4_d)
            cf_sb = consts.tile([128, NBLK * 4], f32)
            nc.sync.dma_start(out=cf_sb, in_=cf_d)
            dfin_sb = consts.tile([KMAX, 3], bf16)
            nc.sync.dma_start(out=dfin_sb, in_=dfin_d)
            adain_sb = consts.tile([3, 9], f32)
            nc.sync.dma_start(out=adain_sb, in_=adain_d)

            # ---- state ----
            x32 = spool.tile([3, P], f32)          # fp32 master of x^T
            state = spool.tile([KMAX, P], bf16)    # rows 0-2: x (bf16); rows 3+3e: k_e
            nc.sync.dma_start(out=x32, in_=x0_d)
            for t in range(NT):
                nc.gpsimd.tensor_copy(out=state[0:3, ts(t, TSZ)], in_=x32[:, ts(t, TSZ)])

            for b in range(NBLK):
                w1s = wpool.tile([KMAX, EV, 4, 128], bf16, tag="w1s")
                nc.sync.dma_start(out=w1s, in_=w1s_d[:, b])
                w23 = wpool.tile([128, 2, 2, 4, 2, 128], fp8, tag="w23")
                nc.sync.dma_start(out=w23, in_=w23_d[:, b])
                w4s = wpool.tile([128, 2, 2, 16], fp8, tag="w4")
                nc.sync.dma_start(out=w4s, in_=w4_d[:, b])

                for e in range(EV):
                    Ke = 3 * (1 + e)
                    h1 = hpool.tile([128, NT, 4, TSW], fp8, tag="h1")
                    h2 = hpool.tile([128, NT, 4, TSW], fp8, tag="h2")
                    h3 = hpool.tile([128, NT, 4, TSW], fp8, tag="h3")
                    ktmp = hpool.tile([3, NT, TSZ], bf16, tag="ktmp")

                    def l1_phase(t):
                        for m in range(4):
                            ps = pspool.tile([128, 512], f32, tag="psL", bufs=6,
                                             name="ps")
                            nc.tensor.matmul(ps[:, :TSZ],
                                             lhsT=w1s[0:Ke, e, m, :],
                                             rhs=state[0:Ke, ts(t, TSZ)],
                                             start=True, stop=True)
                            bias1 = b123_sb[:, b * 12 + m:b * 12 + m + 1]
                            cf1 = cf_sb[:, b * 4 + m:b * 4 + m + 1]
                            if zero_bias:
                                # h1 = relu(ps) * cf in one DVE op
                                nc.vector.tensor_scalar(out=h1[:, t, m, :TSZ],
                                                        in0=ps[:, :TSZ],
                                                        scalar1=0.0, scalar2=cf1,
                                                        op0=Alu.max, op1=Alu.mult)
                            else:
                                r1 = hpool.tile([128, NT, 4, TSW], bf16, tag="r1")
                                nc.scalar.activation(out=r1[:, t, m, :TSZ],
                                                     in_=ps[:, :TSZ],
                                                     func=Act.Relu, bias=bias1)
                                nc.vector.tensor_scalar(out=h1[:, t, m, :TSZ],
                                                        in0=r1[:, t, m, :TSZ],
                                                        scalar1=cf1, scalar2=None,
                                                        op0=Alu.mult)

                    def l23_phase(l, t):
                        hprev, hn = (h1, h2) if l == 0 else (h2, h3)
                        for m in range(4):
                            ps = pspool.tile([128, 512], f32, tag="psL", bufs=6,
                                             name="ps")
                            for kp in range(2):
                                nc.tensor.matmul(ps[:, :TSZ],
                                                 lhsT=w23[:, l, kp, m, :, :],
                                                 rhs=hprev[:, t, 2 * kp:2 * kp + 2, :TSZ],
                                                 start=(kp == 0), stop=(kp == 1),
                                                 perf_mode=DR)
                            bias = b123_sb[:, b * 12 + (l + 1) * 4 + m:b * 12 + (l + 1) * 4 + m + 1]
                            if zero_bias:
                                # h = relu(ps) + h_prev in one DVE op
                                nc.vector.scalar_tensor_tensor(
                                    out=hn[:, t, m, :TSZ], in0=ps[:, :TSZ], scalar=0.0,
                                    in1=hprev[:, t, m, :TSZ], op0=Alu.max, op1=Alu.add)
                            else:
                                r = hpool.tile([128, NT, 4, TSW], bf16,
                                               tag=f"r{l + 2}")
                                nc.scalar.activation(out=r[:, t, m, :TSZ],
                                                     in_=ps[:, :TSZ],
                                                     func=Act.Relu, bias=bias)
                                nc.vector.tensor_tensor(out=hn[:, t, m, :TSZ],
                                                        in0=r[:, t, m, :TSZ],
                                                        in1=hprev[:, t, m, :TSZ],
                                                        op=Alu.add)

                    def l4_phase(t):
                        ps4 = pspool.tile([3, 512], f32, tag="ps4", bufs=2,
                                          name="ps4")
                        for kp in range(2):
                            nc.tensor.matmul(ps4[:, :TSZ],
                                             lhsT=w4s[:, kp, :, 0:3],
                                             rhs=h3[:, t, 2 * kp:2 * kp + 2, :TSZ],
                                             start=(kp == 0), stop=(kp == 1),
                                             perf_mode=DR)
                        nc.scalar.activation(out=ktmp[:, t, :],
                                             in_=ps4[:, :TSZ],
                                             func=Act.Tanh, bias=b4_sb[:, b:b + 1])
                        nc.sync.dma_start(out=state[3 + 3 * e:6 + 3 * e, ts(t, TSZ)],
                                          in_=ktmp[:, t, :])

                    # Diagonal schedule: chain t runs one layer behind chain
                    # t-1, so every dependent phase has >=2 phases of other
                    # chains' matmuls covering its elementwise/DMA tail.
                    for layer, t in ((1, 0), (1, 1), (2, 0), (1, 2), (2, 1),
                                     (3, 0), (2, 2), (3, 1), (4, 0), (4, 1),
                                     (3, 2), (4, 2)):
                        if layer == 1:
                            l1_phase(t)
                        elif layer == 4:
                            l4_phase(t)
                        else:
                            l23_phase(layer - 2, t)

                # block combine: x += sum_j Dfin[j] k_j  (K=51 matmul)
                for t in range(NT):
                    psc = pspool.tile([3, 512], f32, tag="ps4", bufs=2)
                    nc.tensor.matmul(psc[:, :TSZ], lhsT=dfin_sb[:, :],
                                     rhs=state[:, ts(t, TSZ)], start=True, stop=True)
                    nc.vector.tensor_tensor(out=x32[:, ts(t, TSZ)], in0=x32[:, ts(t, TSZ)],
                                            in1=psc[:, :TSZ], op=Alu.add)

                if b % 2 == 1:
                    # AdaIN after each DeformBlock: x = A + M*x - M*mean(x)
                    jj = (b - 1) // 2
                    sums = spool.tile([3, 1], f32, tag="sums")
                    tot = spool.tile([3, 1], f32, tag="tot")
                    tmp = spool.tile([3, 1], f32, tag="tmp")
                    shift = spool.tile([3, 1], f32, tag="shift")
                    nc.vector.reduce_sum(out=sums, in_=x32[:, 0:HALF],
                                         axis=mybir.AxisListType.X)
                    cc_in = dpool.tile([3, 1], f32, tag=f"cc_in{jj}")
                    cc_out = dpool.tile([3, 1], f32, tag=f"cc_out{jj}")
                    nc.sync.dma_start(out=cc_in, in_=sums)
                    nc.gpsimd.collective_compute(
                        "AllReduce", Alu.add, replica_groups=REPLICA_GROUPS,
                        ins=[cc_in.opt()], outs=[cc_out.opt()])
                    nc.sync.dma_start(out=tot, in_=cc_out)
                    # shift = A - (M/N) * total ; x = M*x + shift
                    nc.gpsimd.tensor_tensor(out=tmp, in0=tot,
                                            in1=adain_sb[:, 3 * jj + 2:3 * jj + 3], op=Alu.mult)
                    nc.gpsimd.tensor_tensor(out=shift, in0=adain_sb[:, 3 * jj + 1:3 * jj + 2],
                                            in1=tmp, op=Alu.subtract)
                    for t in range(NT):
                        nc.vector.tensor_scalar(out=x32[:, ts(t, TSZ)],
                                                in0=x32[:, ts(t, TSZ)],
                                                scalar1=adain_sb[:, 3 * jj:3 * jj + 1],
                                                scalar2=shift,
                                                op0=Alu.mult, op1=Alu.add)
                        if b < NBLK - 1:
                            nc.gpsimd.tensor_copy(out=state[0:3, ts(t, TSZ)],
                                                  in_=x32[:, ts(t, TSZ)])
                    nc.sync.dma_start(out=out_d[jj], in_=x32[:, :])
                else:
                    for t in range(NT):
                        nc.gpsimd.tensor_copy(out=state[0:3, ts(t, TSZ)],
                                              in_=x32[:, ts(t, TSZ)])

    nc.compile()
    return nc


def _to_fp8(a):
    return np.clip(np.asarray(a, np.float32), -240.0, 240.0).astype(F8)


def _host_prep(inputs):
    """Host-side preprocessing: shared weights + per-core tensors."""
    clv = np.asarray(inputs["content_latent_vector"], np.float32)   # (B,1,512)
    ap = np.asarray(inputs["adain_params"], np.float32)             # (B,24)
    verts = np.asarray(inputs["vertices"], np.float32)              # (N,3)
    W1 = np.asarray(inputs["W1"], np.float32)
    W2 = np.asarray(inputs["W2"], np.float32)
    W3 = np.asarray(inputs["W3"], np.float32)
    W4 = np.asarray(inputs["W4"], np.float32)
    b1 = np.asarray(inputs["b1"], np.float32)
    b2 = np.asarray(inputs["b2"], np.float32)
    b3 = np.asarray(inputs["b3"], np.float32)
    b4 = np.asarray(inputs["b4"], np.float32)
    Wc = np.asarray(inputs["Wc"], np.float32)
    bc = np.asarray(inputs["bc"], np.float32)
    Wn1 = np.asarray(inputs["Wn1"], np.float32)
    bn1 = np.asarray(inputs["bn1"], np.float32)
    Wn2 = np.asarray(inputs["Wn2"], np.float32)
    bn2 = np.asarray(inputs["bn2"], np.float32)

    C, Dfin = _rk4_coeffs()

    # shared weight packs
    w1s = np.zeros((NBLK, EV, KMAX, H), np.float32)
    for b in range(NBLK):
        for e in range(EV):
            w1s[b, e, 0:3] = W1[b]
            for j in range(e):
                if C[e][j] != 0.0:
                    w1s[b, e, 3 + 3 * j:6 + 3 * j] = C[e][j] * W1[b]
    w1s = (w1s.reshape(NBLK, EV, KMAX, 4, 128)
               .transpose(2, 0, 1, 3, 4)).astype(BF)        # [51,6,16,4,128]

    # DoubleRow pack of W2/W3: [p, b, l, kp, m, i, mm]
    w23 = (np.stack([W2, W3], 1)                            # [b, l, 512, 512]
             .reshape(NBLK, 2, 2, 2, 128, 4, 128)           # [b, l, kp, i, p, m, mm]
             .transpose(4, 0, 1, 2, 5, 3, 6))               # [p, b, l, kp, m, i, mm]
    w23 = np.ascontiguousarray(_to_fp8(w23))

    # DoubleRow pack of W4: [p, b, kp, i, 16] (3 used)
    w4 = (W4.reshape(NBLK, 2, 2, 128, 3)                    # [b, kp, i, p, c]
            .transpose(3, 0, 1, 2, 4))                      # [p, b, kp, i, c]
    w4p = np.zeros((128, NBLK, 2, 2, 16), np.float32)
    w4p[..., 0:3] = w4
    w4p = _to_fp8(w4p)

    b123 = (np.stack([b1, b2, b3], 1)
              .reshape(NBLK, 3, 4, 128)
              .transpose(3, 0, 1, 2)
              .reshape(128, NBLK * 12)).astype(np.float32).copy()      # [128, 6*3*4]
    b4p = b4.T.astype(np.float32).copy()                               # [3,6]
    dfin = np.zeros((KMAX, 3), np.float32)
    for j in range(EV):
        for i in range(3):
            dfin[3 + 3 * j + i, i] = Dfin[j]
    dfin = dfin.astype(BF)

    def sigmoid(x):
        return 1.0 / (1.0 + np.exp(-x))

    # conditioning features per block: (6, B, 512)
    cf_all = np.stack([np.tanh(clv @ Wc[k] + bc[k])[:, 0, :] for k in range(NBLK)])

    # AdaIN affine constants per j (including initial j=0 applied on host)
    adain_M = np.zeros((4, B, 3), np.float32)
    adain_A = np.zeros((4, B, 3), np.float32)
    for j in range(4):
        p6 = ap[:, 6 * j:6 * j + 6]
        scale = sigmoid(np.maximum(clv @ Wn1[j] + bn1[j], 0.0) @ Wn2[j] + bn2[j])[:, 0, :]
        adain_M[j] = p6[:, 3:] * (1.0 - scale)
        adain_A[j] = p6[:, :3]

    # initial AdaIN on host: x0 = A0 + M0*(verts - mean(verts)) per batch
    vmean = verts.mean(0)
    x0_full = (adain_A[0][:, None, :]
               + adain_M[0][:, None, :] * (verts[None] - vmean[None, None]))  # (B,N,3)

    shared = {"w1s": w1s, "w23": w23, "w4": w4p, "b123": b123,
              "b4": b4p, "dfin": dfin}

    in_maps = []
    for c in range(8):
        bidx, half = c // 2, c % 2
        xc = np.zeros((3, P), np.float32)
        xc[:, :HALF] = x0_full[bidx, half * HALF:(half + 1) * HALF].T
        cfc = cf_all[:, bidx, :].reshape(NBLK, 4, 128).transpose(2, 0, 1).reshape(128, NBLK * 4)
        adain_c = np.zeros((3, 9), np.float32)
        for j in range(1, 4):
            adain_c[:, 3 * (j - 1) + 0] = adain_M[j][bidx]
            adain_c[:, 3 * (j - 1) + 1] = adain_A[j][bidx]
            adain_c[:, 3 * (j - 1) + 2] = adain_M[j][bidx] / np.float32(N_FULL)
        m = dict(shared)
        m["x0"] = xc
        m["cf"] = np.ascontiguousarray(cfc.astype(np.float32))
        m["adain"] = adain_c
        in_maps.append(m)
    return in_maps


def kernel(**inputs) -> np.ndarray:
    global LAST_RESULTS
    from concourse.bass_utils import run_bass_kernel_spmd

    zero_bias = all(
        not np.any(np.asarray(inputs[k]))
        for k in ("b1", "b2", "b3", "b4"))
    key = ("nc", zero_bias)
    if key not in _CACHE:
        _CACHE[key] = _build_bass(zero_bias)
    nc = _CACHE[key]

    in_maps = _host_prep(inputs)
    res = run_bass_kernel_spmd(nc, in_maps, core_ids=list(range(8)), trace=TRACE)
    LAST_RESULTS = res

    full = np.zeros((3, B, N_FULL, 3), np.float32)
    for c in range(8):
        bidx, half = c // 2, c % 2
        chunk = res.results[c]["out"][:, :, :HALF]          # (3, 3ch, HALF)
        full[:, bidx, half * HALF:(half + 1) * HALF, :] = chunk.transpose(0, 2, 1)
    return full
